# revision 1
# baseline (speedup 1.0000x reference)
"""BotGraphSAGE on 8 TRN2 NeuronCores (Bass/Tile).

Strategy (graph/data parallel, node sharding):
  - Relabel nodes by ascending degree into "dev" ids padded to 50176;
    global stripe k = devs [1024k, 1024(k+1)) (near-equal degree);
    core p owns dev = 1024k + 128p + q  (q=0..127) => every core's k-th
    128-node group has the same max degree -> shared slot counts (SPMD).
  - T-row (row in the all-gathered feature table) of a dev:
    trow = p*6272 + k*128 + q.
  - Per layer: z = x @ Wl computed on the owning core (feature-major
    matmuls), AllGather z -> T [50176, d]; neighbor sums via dma_gather
    (int16 idx) from two overlapping views T[0:32768) / T[17408:50176)
    with per-node balanced edge assignment; pad slots point at virtual
    (all-zero) rows.  Aggregation = identity-matmul accumulation in PSUM
    (float32r), then mean-scale + root term + bias + relu.
  - Everything flows in float32r (fp32 bits, PE rounds ~13 mantissa bits).
"""

import os

import numpy as np

import concourse.bass as bass
import concourse.tile as tile
from concourse import bacc, mybir
from concourse.bass_utils import run_bass_kernel_spmd

# ---------------- constants ----------------
N = 50000
E = 1600000
C = 8
NP = 50176          # padded node count = 8 * 6272
SH = 6272           # nodes per core
NSTR = 49           # stripes (128-node groups per core)
STRIPE = 1024
LOB = 32768         # lo table = T[0:LOB)
HIO = 17408         # hi table = T[HIO:NP)
CAP = int(os.environ.get("KCAP", "24"))  # max slots per dma_gather call
LAYERS = [(128, 64), (64, 128), (128, 64), (64, 128)]
F32R = mybir.dt.float32r
F32 = mybir.dt.float32
I16 = mybir.dt.int16

_CACHE = {}
LAST_EXEC_NS = None


def _roundup4(x):
    return (int(x) + 3) & ~3


def _trow_of_dev_table():
    dev = np.arange(NP)
    k = dev // STRIPE
    j = dev % STRIPE
    p = j // 128
    q = j % 128
    return p * SH + k * 128 + q


def _host_prep(edge_index):
    src = np.asarray(edge_index[0], np.int64)
    dst = np.asarray(edge_index[1], np.int64)

    deg = np.bincount(dst, minlength=N)
    order = np.argsort(deg, kind="stable")  # real ids, ascending degree

    # virtual (zero) devs: 175 low ones + dev 39168 (trow exactly HIO)
    virt = np.concatenate([np.arange(175), np.array([39168])])
    is_virt = np.zeros(NP, bool)
    is_virt[virt] = True
    real_slots = np.flatnonzero(~is_virt)
    assert real_slots.size == N
    old_of_dev = np.full(NP, -1, np.int64)
    old_of_dev[real_slots] = order
    dev_of_old = np.empty(N, np.int64)
    dev_of_old[order] = real_slots

    trow_of_dev = _trow_of_dev_table()
    assert trow_of_dev[0] == 0 and trow_of_dev[39168] == HIO

    srcd = dev_of_old[src]
    dstd = dev_of_old[dst]
    trow_src = trow_of_dev[srcd]

    degd = np.bincount(dstd, minlength=NP)
    inv_dev = (1.0 / np.maximum(degd, 1)).astype(np.float32)

    # per-edge class by source trow: 0 = lo-only, 1 = flexible, 2 = hi-only
    cls = np.where(trow_src < HIO, 0, np.where(trow_src >= LOB, 2, 1)).astype(np.int8)
    na = np.bincount(dstd[cls == 0], minlength=NP)
    nf = np.bincount(dstd[cls == 1], minlength=NP)
    tA = (degd + 1) // 2
    cutA = na + np.clip(tA - na, 0, nf)          # |A| per dev
    cntB = degd - cutA

    # order edges by (dst dev, class); rank within node
    eo = np.lexsort((cls, dstd))
    dstd_s = dstd[eo]
    trow_s = trow_src[eo]
    starts = np.zeros(NP + 1, np.int64)
    starts[1:] = np.cumsum(degd)
    r = np.arange(E) - starts[dstd_s]
    inA = r < cutA[dstd_s]
    slotA = r
    slotB = r - cutA[dstd_s]

    # per-stripe shared slot counts
    cutA_st = cutA.reshape(NSTR, STRIPE)
    cntB_st = cntB.reshape(NSTR, STRIPE)
    L_A = [_roundup4(cutA_st[k].max()) for k in range(NSTR)]
    L_B = [_roundup4(cntB_st[k].max()) for k in range(NSTR)]
    offA = np.concatenate([[0], np.cumsum(L_A)]).astype(np.int64)
    offB = np.concatenate([[0], np.cumsum(L_B)]).astype(np.int64)
    SLA, SLB = int(offA[-1]), int(offB[-1])

    # grids: [C, SL, 128] int16, pads point at zero rows (local idx 0)
    k_e = dstd_s // STRIPE
    p_e = (dstd_s % STRIPE) // 128
    q_e = dstd_s % 128
    gridA = np.zeros((C, SLA, 128), np.int16)
    gridB = np.zeros((C, SLB, 128), np.int16)
    mA = inA
    assert trow_s[mA].max() < LOB
    gridA[p_e[mA], offA[k_e[mA]] + slotA[mA], q_e[mA]] = trow_s[mA].astype(np.int16)
    mB = ~inA
    hi_local = trow_s[mB] - HIO
    assert hi_local.min() >= 0 and hi_local.max() <= 32767
    gridB[p_e[mB], offB[k_e[mB]] + slotB[mB], q_e[mB]] = hi_local.astype(np.int16)

    def wrap16(flat):
        # call position i -> [i % 16, i // 16], replicated to 128 partitions
        return np.ascontiguousarray(
            np.tile(flat.reshape(-1, 16).T, (8, 1))
        )

    idxa = [wrap16(gridA[p].reshape(-1)) for p in range(C)]
    idxb = [wrap16(gridB[p].reshape(-1)) for p in range(C)]

    # inv_cnt per core: [128, NSTR]
    kk = np.arange(NSTR)
    qq = np.arange(128)
    inv_core = []
    dev_of_loc = []
    for p in range(C):
        dev_pk = (STRIPE * kk[None, :] + 128 * p + qq[:, None])  # [128, NSTR]
        inv_core.append(inv_dev[dev_pk].astype(np.float32))
        dev_of_loc.append((STRIPE * kk[:, None] + 128 * p + qq[None, :]).reshape(-1))

    meta = dict(L_A=L_A, L_B=L_B, offA=offA, offB=offB, SLA=SLA, SLB=SLB)
    return meta, idxa, idxb, inv_core, dev_of_loc, old_of_dev


def _build_nc(meta):
    stages = int(os.environ.get("KSTAGES", "6"))
    L_A, L_B = meta["L_A"], meta["L_B"]
    offA, offB = meta["offA"], meta["offB"]
    SLA, SLB = meta["SLA"], meta["SLB"]

    nc = bacc.Bacc(None, target_bir_lowering=False)

    def din(name, shape, dt=F32R):
        return nc.dram_tensor(name, shape, dt, kind="ExternalInput")

    dest_in = din("dest", [128, 6 * SH])          # des^T k-tiled
    numt_in = din("numt", [4, SH])
    catt_in = din("catt", [3, SH])
    idxa_in = din("idxa", [128, max(SLA * 8, 16)], I16)
    idxb_in = din("idxb", [128, max(SLB * 8, 16)], I16)
    inv_in = din("inv", [128, NSTR], F32)
    ident_in = din("ident", [128, 128])
    ones_in = din("ones1", [1, 128])
    wdes_in = din("wdes", [128, 6 * 32])
    wnum_in = din("wnum", [4, 42])
    wcat_in = din("wcat", [3, 42])
    wind_in = din("wind", [32, 128])
    winn_in = din("winn", [42, 128])
    winc_in = din("winc", [42, 128])
    wl_in = [din(f"wl{i}", [di, do]) for i, (di, do) in enumerate(LAYERS)]
    wr_in = [din(f"wr{i}", [di, do]) for i, (di, do) in enumerate(LAYERS)]
    bl_in = [din(f"bl{i}", [1, do]) for i, (_, do) in enumerate(LAYERS)]
    wo1_in = din("wo1", [128, 128])
    wo2_in = din("wo2", [128, 2])
    bdes_in = din("bdes", [32, 1], F32)
    bnum_in = din("bnum", [42, 1], F32)
    bcat_in = din("bcat", [42, 1], F32)
    bin_in = din("bin", [128, 1], F32)
    bo1_in = din("bo1", [128, 1], F32)
    bo2_in = din("bo2", [2, 1], F32)
    out_ext = nc.dram_tensor("out", [2, SH], F32, kind="ExternalOutput")

    AF = mybir.ActivationFunctionType
    OP = mybir.AluOpType

    with tile.TileContext(nc) as tc:
        with (
            tc.tile_pool(name="res", bufs=1) as res,
            tc.tile_pool(name="dram", bufs=1, space="DRAM") as dram,
        ):
            # ---- resident loads ----
            def load(pool, src_ap, shape, dt, tag):
                t = pool.tile(shape, dt, tag=tag)
                nc.sync.dma_start(t[:], src_ap)
                return t

            idxa_t = load(res, idxa_in[:], [128, max(SLA * 8, 16)], I16, "idxa")
            idxb_t = load(res, idxb_in[:], [128, max(SLB * 8, 16)], I16, "idxb")
            inv_t = load(res, inv_in[:], [128, NSTR], F32, "inv")
            ident_t = load(res, ident_in[:], [128, 128], F32R, "ident")
            ones_t = load(res, ones_in[:], [1, 128], F32R, "ones")
            wdes_t = load(res, wdes_in[:], [128, 6 * 32], F32R, "wdes")
            wnum_t = load(res, wnum_in[:], [4, 42], F32R, "wnum")
            wcat_t = load(res, wcat_in[:], [3, 42], F32R, "wcat")
            wind_t = load(res, wind_in[:], [32, 128], F32R, "wind")
            winn_t = load(res, winn_in[:], [42, 128], F32R, "winn")
            winc_t = load(res, winc_in[:], [42, 128], F32R, "winc")
            wl_t = [load(res, wl_in[i][:], list(wl_in[i].shape), F32R, f"wl{i}") for i in range(4)]
            wr_t = [load(res, wr_in[i][:], list(wr_in[i].shape), F32R, f"wr{i}") for i in range(4)]
            bl_t = [load(res, bl_in[i][:], list(bl_in[i].shape), F32R, f"bl{i}") for i in range(4)]
            wo1_t = load(res, wo1_in[:], [128, 128], F32R, "wo1")
            wo2_t = load(res, wo2_in[:], [128, 2], F32R, "wo2")
            bdes_t = load(res, bdes_in[:], [32, 1], F32, "bdes")
            bnum_t = load(res, bnum_in[:], [42, 1], F32, "bnum")
            bcat_t = load(res, bcat_in[:], [42, 1], F32, "bcat")
            bin_t = load(res, bin_in[:], [128, 1], F32, "bin")
            bo1_t = load(res, bo1_in[:], [128, 1], F32, "bo1")
            bo2_t = load(res, bo2_in[:], [2, 1], F32, "bo2")

            xt_a = res.tile([128, SH], F32R, tag="xta")
            xt_b = res.tile([128, SH], F32R, tag="xtb")

            # chunking of the 6272 node columns
            chunks = [(c * 512, min(512, SH - c * 512)) for c in range((SH + 511) // 512)]

            # ---- stage 0: features -> x0^T (into xt_a) ----
            with (
                tc.tile_pool(name="s0", bufs=2) as s0,
                tc.tile_pool(name="s0p", bufs=2, space="PSUM") as s0p,
            ):
                for c0, w in chunks:
                    dch = s0.tile([128, 6, 512], F32R, tag="dch")
                    nc.sync.dma_start(
                        dch[:, :, :w],
                        dest_in[:].rearrange("p (t n) -> p t n", t=6)[:, :, c0 : c0 + w],
                    )
                    nch = s0.tile([4, 512], F32R, tag="nch")
                    nc.sync.dma_start(nch[:, :w], numt_in[:, c0 : c0 + w])
                    cch = s0.tile([3, 512], F32R, tag="cch")
                    nc.sync.dma_start(cch[:, :w], catt_in[:, c0 : c0 + w])

                    ps_d = s0p.tile([32, 512], F32, tag="psd")
                    for t in range(6):
                        nc.tensor.matmul(
                            ps_d[:, :w],
                            lhsT=wdes_t[:, t * 32 : (t + 1) * 32],
                            rhs=dch[:, t, :w],
                            start=(t == 0),
                            stop=(t == 5),
                        )
                    ps_n = s0p.tile([42, 512], F32, tag="psn")
                    nc.tensor.matmul(ps_n[:, :w], lhsT=wnum_t[:], rhs=nch[:, :w])
                    ps_c = s0p.tile([42, 512], F32, tag="psc")
                    nc.tensor.matmul(ps_c[:, :w], lhsT=wcat_t[:], rhs=cch[:, :w])

                    dT = s0.tile([32, 512], F32R, tag="dT")
                    nc.scalar.activation(dT[:, :w], ps_d[:, :w], AF.Lrelu, bias=bdes_t[:, :1], alpha=0.01)
                    nT = s0.tile([42, 512], F32R, tag="nT")
                    nc.scalar.activation(nT[:, :w], ps_n[:, :w], AF.Lrelu, bias=bnum_t[:, :1], alpha=0.01)
                    cT = s0.tile([42, 512], F32R, tag="cT")
                    nc.scalar.activation(cT[:, :w], ps_c[:, :w], AF.Lrelu, bias=bcat_t[:, :1], alpha=0.01)

                    ps_x = s0p.tile([128, 512], F32, tag="psx")
                    nc.tensor.matmul(ps_x[:, :w], lhsT=wind_t[:], rhs=dT[:, :w], start=True, stop=False)
                    nc.tensor.matmul(ps_x[:, :w], lhsT=winn_t[:], rhs=nT[:, :w], start=False, stop=False)
                    nc.tensor.matmul(ps_x[:, :w], lhsT=winc_t[:], rhs=cT[:, :w], start=False, stop=True)
                    nc.scalar.activation(xt_a[:, c0 : c0 + w], ps_x[:, :w], AF.Lrelu, bias=bin_t[:, :1], alpha=0.01)

            # ---- SAGE layers ----
            nlayers = max(0, min(4, stages - 2 + 1)) if stages < 6 else 4
            with (
                tc.tile_pool(name="sg", bufs=2) as sg,
                tc.tile_pool(name="gza", bufs=2) as gza,
                tc.tile_pool(name="gzb", bufs=2) as gzb,
                tc.tile_pool(name="zshp", bufs=1) as zshp,
                tc.tile_pool(name="pz", bufs=2, space="PSUM") as pz,
                tc.tile_pool(name="pr", bufs=2, space="PSUM") as pr,
                tc.tile_pool(name="ps", bufs=2, space="PSUM") as ps_pool,
                tc.tile_pool(name="pt", bufs=2, space="PSUM") as pt,
            ):
                for li, (din_, dout) in enumerate(LAYERS[:nlayers]):
                    xt_in = xt_a if li % 2 == 0 else xt_b
                    xt_out = xt_b if li % 2 == 0 else xt_a
                    PACK = 512 // dout
                    wl, wr, bl = wl_t[li], wr_t[li], bl_t[li]

                    # dense z = x @ Wl (node-major rows), staged to HBM
                    zsh = zshp.tile([128, NSTR * dout], F32R, tag="zsh")
                    for k in range(NSTR):
                        ps_z = pz.tile([128, dout], F32, tag="psz")
                        nc.tensor.matmul(
                            ps_z[:],
                            lhsT=xt_in[:din_, k * 128 : (k + 1) * 128],
                            rhs=wl[:],
                        )
                        nc.scalar.copy(zsh[:, k * dout : (k + 1) * dout], ps_z[:])
                    z_dram = dram.tile([SH, dout], F32R, tag=f"zd{li}")
                    nc.sync.dma_start(
                        z_dram[:].rearrange("(k q) f -> q k f", q=128),
                        zsh[:].rearrange("q (k f) -> q k f", k=NSTR),
                    )
                    T = dram.tile([NP, dout], F32R, tag=f"T{li}")
                    nc.gpsimd.collective_compute(
                        "AllGather",
                        mybir.AluOpType.bypass,
                        replica_groups=[list(range(C))],
                        ins=[z_dram[:].opt()],
                        outs=[T[:].opt()],
                    )
                    T_lo = T[0:LOB, :]
                    T_hi = T[HIO:NP, :]

                    for k in range(NSTR):
                        LA, LB = L_A[k], L_B[k]
                        # root term + bias
                        ps_r = pr.tile([128, dout], F32, tag="psr")
                        nc.tensor.matmul(
                            ps_r[:],
                            lhsT=xt_in[:din_, k * 128 : (k + 1) * 128],
                            rhs=wr[:],
                            start=True,
                            stop=False,
                        )
                        nc.tensor.matmul(
                            ps_r[:], lhsT=ones_t[:], rhs=bl[:], start=False, stop=True
                        )

                        if LA + LB > 0:
                            wb = min(max(LA, LB), PACK)
                            ps_s = ps_pool.tile([128, 512], F32, tag="pss")
                            grids = [
                                (LA, offA[k], idxa_t, T_lo, gza, "ga"),
                                (LB, offB[k], idxb_t, T_hi, gzb, "gb"),
                            ]
                            grids.sort(key=lambda g: -min(g[0], PACK))
                            n_mm = sum(
                                -(-min(CAP, L - s0x) // PACK)
                                for (L, _, _, _, _, _) in grids
                                for s0x in range(0, L, CAP)
                            )
                            mi = 0
                            for L, off, idxt, tab, gp, gtag in grids:
                                for s0x in range(0, L, CAP):
                                    sl = min(CAP, L - s0x)
                                    g = gp.tile([128, CAP, dout], F32R, tag=gtag)
                                    nc.gpsimd.dma_gather(
                                        out_ap=g[:, :sl, :],
                                        in_ap=tab,
                                        idxs_ap=idxt[
                                            :, (off + s0x) * 8 : (off + s0x + sl) * 8
                                        ],
                                        num_idxs=sl * 128,
                                        num_idxs_reg=sl * 128,
                                        elem_size=dout,
                                        single_packet=os.environ.get("KSP", "0") == "1",
                                    )
                                    for pp in range(0, sl, PACK):
                                        wd = min(PACK, sl - pp)
                                        nc.tensor.matmul(
                                            ps_s[:, : wd * dout],
                                            lhsT=ident_t[:],
                                            rhs=g[:, pp : pp + wd, :],
                                            start=(mi == 0),
                                            stop=(mi == n_mm - 1),
                                        )
                                        mi += 1
                            # fold + mean + root + relu
                            tmp = sg.tile([128, 128], F32, tag="tmp")
                            nc.scalar.copy(tmp[:, :dout], ps_s[:, :dout])
                            for j in range(1, wb):
                                nc.vector.tensor_add(
                                    tmp[:, :dout],
                                    tmp[:, :dout],
                                    ps_s[:, j * dout : (j + 1) * dout],
                                )
                            nc.scalar.activation(
                                tmp[:, :dout], tmp[:, :dout], AF.Copy,
                                scale=inv_t[:, k : k + 1],
                            )
                            nc.vector.tensor_add(tmp[:, :dout], tmp[:, :dout], ps_r[:])
                            hsb = sg.tile([128, 128], F32R, tag="hsb")
                            nc.scalar.activation(hsb[:, :dout], tmp[:, :dout], AF.Relu)
                        else:
                            hsb = sg.tile([128, 128], F32R, tag="hsb")
                            nc.scalar.activation(hsb[:, :dout], ps_r[:], AF.Relu)

                        ps_t = pt.tile([128, 128], F32R, tag="pst")
                        nc.tensor.transpose(
                            ps_t[:dout, :], hsb[:, :dout], ident_t[:]
                        )
                        nc.scalar.copy(
                            xt_out[:dout, k * 128 : (k + 1) * 128], ps_t[:dout, :]
                        )

            # ---- final dense stack ----
            with (
                tc.tile_pool(name="fin", bufs=2) as fin,
                tc.tile_pool(name="finp", bufs=2, space="PSUM") as finp,
            ):
                outT = res.tile([2, SH], F32, tag="outT")
                xt4 = xt_a  # after 4 layers output lands back in xt_a
                if stages < 6:
                    nc.vector.memset(outT[:], 0.0)
                    nc.gpsimd.dma_start(out_ext[:], outT[:])
                    chunks_f = []
                else:
                    chunks_f = chunks
                for c0, w in chunks_f:
                    ps1 = finp.tile([128, 512], F32, tag="ps1")
                    nc.tensor.matmul(ps1[:, :w], lhsT=wo1_t[:], rhs=xt4[:, c0 : c0 + w])
                    xo = fin.tile([128, 512], F32R, tag="xo")
                    nc.scalar.activation(xo[:, :w], ps1[:, :w], AF.Lrelu, bias=bo1_t[:, :1], alpha=0.01)
                    ps2 = finp.tile([2, 512], F32, tag="ps2")
                    nc.tensor.matmul(ps2[:, :w], lhsT=wo2_t[:], rhs=xo[:, :w])
                    nc.scalar.activation(
                        outT[:, c0 : c0 + w], ps2[:, :w], AF.Identity, bias=bo2_t[:, :1]
                    )
                nc.sync.dma_start(out_ext[:], outT[:])

    nc.finalize()
    return nc


def _pack_weights(inputs):
    f = lambda a: np.ascontiguousarray(np.asarray(a, np.float32))
    W_des = f(inputs["W_des"])          # [768, 32]
    wdes = np.ascontiguousarray(
        W_des.reshape(6, 128, 32).transpose(1, 0, 2).reshape(128, 192)
    )
    W_in = f(inputs["W_in"])            # [116, 128]
    w = dict(
        wdes=wdes,
        wnum=f(inputs["W_num"]),
        wcat=f(inputs["W_cat"]),
        wind=np.ascontiguousarray(W_in[0:32]),
        winn=np.ascontiguousarray(W_in[32:74]),
        winc=np.ascontiguousarray(W_in[74:116]),
        wo1=f(inputs["W_o1"]),
        wo2=f(inputs["W_o2"]),
        ones1=np.ones((1, 128), np.float32),
        ident=np.eye(128, dtype=np.float32),
        bdes=f(inputs["b_des"]).reshape(32, 1),
        bnum=f(inputs["b_num"]).reshape(42, 1),
        bcat=f(inputs["b_cat"]).reshape(42, 1),
        bin=f(inputs["b_in"]).reshape(128, 1),
        bo1=f(inputs["b_o1"]).reshape(128, 1),
        bo2=f(inputs["b_o2"]).reshape(2, 1),
    )
    for i, nm in enumerate(["s1a", "s1b", "s2a", "s2b"]):
        w[f"wl{i}"] = f(inputs[f"{nm}_Wl"])
        w[f"wr{i}"] = f(inputs[f"{nm}_Wr"])
        w[f"bl{i}"] = f(inputs[f"{nm}_bl"]).reshape(1, -1)
    # virtual zero rows rely on zero biases (see module docstring)
    for b in ["bdes", "bnum", "bcat", "bin"] + [f"bl{i}" for i in range(4)]:
        assert not np.any(w[b]), "nonzero biases break the virtual-zero-row padding"
    return w


def kernel(**inputs):
    edge_index = np.asarray(inputs["edge_index"])
    key = (hash(edge_index.tobytes()), os.environ.get("KSTAGES", "6"), CAP, os.environ.get("KSP", "1"))
    if key in _CACHE:
        nc, meta, idxa, idxb, inv_core, dev_of_loc, old_of_dev = _CACHE[key]
    else:
        meta, idxa, idxb, inv_core, dev_of_loc, old_of_dev = _host_prep(edge_index)
        nc = _build_nc(meta)
        _CACHE[key] = (nc, meta, idxa, idxb, inv_core, dev_of_loc, old_of_dev)

    w = _pack_weights(inputs)

    des = np.asarray(inputs["des"], np.float32)
    nump = np.asarray(inputs["num_prop"], np.float32)
    catp = np.asarray(inputs["cat_prop"], np.float32)

    in_maps = []
    for p in range(C):
        devs = dev_of_loc[p]
        olds = old_of_dev[devs]
        valid = olds >= 0
        oc = np.where(valid, olds, 0)

        des_c = des[oc]
        des_c[~valid] = 0.0
        dest = np.ascontiguousarray(
            des_c.T.reshape(6, 128, SH).transpose(1, 0, 2).reshape(128, 6 * SH)
        )
        num_c = nump[oc]
        num_c[~valid] = 0.0
        cat_c = catp[oc]
        cat_c[~valid] = 0.0

        m = dict(
            dest=dest,
            numt=np.ascontiguousarray(num_c.T),
            catt=np.ascontiguousarray(cat_c.T),
            idxa=idxa[p],
            idxb=idxb[p],
            inv=inv_core[p],
            **w,
        )
        in_maps.append(m)

    trace = os.environ.get("BASS_KERNEL_TRACE") == "1"
    res = run_bass_kernel_spmd(nc, in_maps, core_ids=list(range(C)), trace=trace)
    global LAST_EXEC_NS
    LAST_EXEC_NS = res.exec_time_ns

    out = np.zeros((N, 2), np.float32)
    for p in range(C):
        devs = dev_of_loc[p]
        olds = old_of_dev[devs]
        valid = olds >= 0
        vals = res.results[p]["out"].T  # [6272, 2]
        out[olds[valid]] = vals[valid]
    return out

